# revision 58
# baseline (speedup 1.0000x reference)
"""Trainium2 Bass kernel for nn_MoETransformerBlock (MoE-LoRA ViT block).

Strategy: data-parallel over batch across 8 NeuronCores (2 batch elems per
core), weights replicated, no collectives. Activations are feature-major
[feature, token] in SBUF. LayerNorm-1 is *folded* into the consumers: the
QKV/gate/LoRA matmuls run on raw x, a K=1 correction-row matmul subtracts
mean*colsum(W), and the per-token rstd is applied at PSUM eviction
(tensor_mul with a broadcast row / per-partition column). LN2 is explicit
but chunk-pipelined behind the proj matmuls. Softmax denominators come from
a ones-augmented V; reciprocals use the fast-approx DVE op; exp ACTs are
merged pairwise (two heads per ACTIVATE) to halve ScalarE overhead. FC1/FC2
weights stream in quarters (double-buffered), FC2 accumulates across
quarters via a bf16 partial buffer.
"""

import sys

sys.path.insert(0, "/opt/trn_rl_repo")

import numpy as np
import ml_dtypes

BF16 = ml_dtypes.bfloat16

# ---- problem constants (hardcoded; must match reference.py) ----
B, N, E, H, HD = 16, 577, 1024, 16, 64
LORA_E, LORA_R = 4, 16
AD_E, AD_D = 4, 64
FF = 4 * E
NCORES = 8
BLOC = B // NCORES        # 2 batch elems per core
T = BLOC * N              # 1154 tokens per core
NKT = E // 128            # 8 feature k-tiles
NQT = 5                   # token tiles per batch: 4x128 + 65
NFQ = 4                   # fc weight quarters
FH = FF // NFQ // 128     # 8 fc1-Mtiles (= fc2-ktiles) per quarter

# per-batch token tiles (global token start, size)
TOKTILES = [(b * N + i * 128, min(128, N - i * 128))
            for b in range(BLOC) for i in range(NQT)]
NTT = len(TOKTILES)       # 10
# free-dim chunks (global token start, size) for batch-agnostic ops
CHUNKS = [(0, 512), (512, 512), (1024, 130)]
QCHUNKS = [(0, 512), (512, 65)]  # batch-local (attention)


def _build(tc, io, cfg):
    """Emit the Tile program. io: dict name -> bass.AP (dram)."""
    import concourse.bass as bass
    import concourse.mybir as mybir
    from concourse.masks import make_identity

    nc = tc.nc
    f32 = mybir.dt.float32
    bf = mybir.dt.bfloat16
    AF = mybir.ActivationFunctionType
    OP = mybir.AluOpType

    def mm(out, lhsT, rhs, start, stop):
        nc.tensor.matmul(out, lhsT, rhs, start=start, stop=stop)

    import contextlib
    ctx = contextlib.ExitStack()
    with ctx:
        sp = ctx.enter_context(tc.tile_pool(name="persist", bufs=1))

        # ---------- persistent SBUF (phase-1/2 lifetime things) ----------
        x_sb = sp.tile([128, NKT, T], bf)          # tokens -> t1 residual
        xr = io["x_fm"].rearrange("(k p) t -> p k t", p=128)
        for (cs, cn) in CHUNKS:
            nc.sync.dma_start(x_sb[:, :, cs:cs + cn], xr[:, :, cs:cs + cn])

        ident = sp.tile([128, 128], f32)
        make_identity(nc, ident)
        ones_c = sp.tile([128, 1], bf)             # column of ones (colsum lhsT)
        nc.vector.memset(ones_c, 1.0)
        eps_t = sp.tile([1, 1], f32)
        nc.vector.memset(eps_t, 1e-6)

        atz_sb = sp.tile([128, NKT, 68], bf)       # [lora_A.T | lora_gate.T]
        nc.sync.dma_start(atz_sb, io["atz"].rearrange("(k p) c -> p k c", p=128))
        negrs_sb = sp.tile([1, 68], bf)
        nc.sync.dma_start(negrs_sb, io["negrs"])
        negcsqk_sb = sp.tile([1, 2 * E], bf)
        nc.sync.dma_start(negcsqk_sb, io["negcsqk"])
        negcsv_sb = sp.tile([1, E], bf)
        nc.sync.dma_start(negcsv_sb, io["negcsv"])
        bqkvT_sb = sp.tile([64, 3 * E], bf)
        nc.sync.dma_start(bqkvT_sb, io["bqkvT"])
        elora_sb = sp.tile([4, 64], bf)
        nc.sync.dma_start(elora_sb, io["elora"])
        bp_sb = sp.tile([128, 8], f32)
        nc.sync.dma_start(bp_sb, io["bp"].rearrange("(m p) -> p m", p=128))
        wgad_sb = sp.tile([128, NKT, 4], bf)
        nc.sync.dma_start(wgad_sb, io["wgad"].rearrange("(k p) c -> p k c", p=128))
        if cfg["has_bqk"]:
            bqk_sb = sp.tile([128, 16], f32)
            nc.sync.dma_start(bqk_sb, io["bqk"].rearrange("(m p) -> p m", p=128))
        if cfg["has_bh"]:
            bh_sb = sp.tile([64, 1], f32)
            nc.sync.dma_start(bh_sb, io["bh"].rearrange("(p o) -> p o", o=1))
        if cfg["has_bgl"]:
            bgl_sb = sp.tile([4, 1], f32)
            nc.sync.dma_start(bgl_sb, io["bgl"].rearrange("(p o) -> p o", o=1))
        if cfg["has_bgad"]:
            bgad_sb = sp.tile([4, 1], f32)
            nc.sync.dma_start(bgad_sb, io["bgad"].rearrange("(p o) -> p o", o=1))

        # mid-lifetime: LN2 outputs, written in phase 2, read in phase 3
        mid = ctx.enter_context(tc.tile_pool(name="mid", bufs=1))
        n2 = mid.tile([128, NKT, T], bf, tag="n2")

        # attention-lifetime buffers (stack slot reserved early; freed after
        # proj so phase 3 can reuse the space)
        actx = contextlib.ExitStack()
        ap_ = actx.enter_context(tc.tile_pool(name="attn_bufs", bufs=1))
        qk_sb = ap_.tile([128, 16, T], bf)
        v_sb = ap_.tile([128, 2 * NQT, H * 65], bf)
        nc.vector.memset(
            v_sb.rearrange("p t (h c) -> p t h c", c=65)[:, :, :, 64:65], 1.0)
        ctx_sb = ap_.tile([128, NKT, T], bf)
        if cfg["has_bv"]:
            bv_bc = ap_.tile([128, E], f32)
            nc.sync.dma_start(bv_bc, io["bv"].to_broadcast((128, E)))

        # proj weight: lives until end of proj (DMA emitted later, after wv/wqk)
        wpctx = contextlib.ExitStack()
        wpp = wpctx.enter_context(tc.tile_pool(name="wpp", bufs=1))
        wp_sb = wpp.tile([128, NKT, E], bf)

        # phase-1 rows (LN1 stats) + lora gh: live until end of attention
        p1ctx = contextlib.ExitStack()
        rp1 = p1ctx.enter_context(tc.tile_pool(name="rp1", bufs=1))
        gh_lora = rp1.tile([64, T], bf, tag="gh_lora")

        # qkv weight: lives until end of attention (DMA emitted later)
        wqkctx = contextlib.ExitStack()
        p1w = wqkctx.enter_context(tc.tile_pool(name="p1w", bufs=1))
        wqk_sb = p1w.tile([128, NKT, 2048], bf)

        # ================= LN-stats helper (rows + broadcasts) =================
        def ln_stats(src, rp, tmp, xp, pp, want_cols, keep_std):
            """Colsum-based stats of feature-major src. Returns dict with
            m_bf [1,T] bf16, r_bc [128,T] bf16, rcol [128,NTT] f32.
            After sqrt, rstd overwrites the var row; std lives in std_f."""
            st = {}
            rows = tmp.tile([1, 2, T], f32, tag="ln_trows")
            mean_f = rows[:, 0, :]
            var_f = rows[:, 1, :]
            rstd_f = var_f                          # recip overwrites var
            std_f = (rp if keep_std else tmp).tile([1, T], f32, tag="std_f")
            m_bf = rp.tile([1, T], bf, tag="m_bf")
            r_bf = tmp.tile([1, T], bf, tag="r_bf")
            r_bc = rp.tile([128, T], bf, tag="r_bc")
            for (cs, cn) in CHUNKS:
                ssq = pp.tile([1, 2, 512], f32, tag="ln_rows")
                for kt in range(NKT):
                    mm(ssq[:, 0, :cn], ones_c, src[:, kt, cs:cs + cn],
                       start=(kt == 0), stop=(kt == NKT - 1))
                for kt in range(NKT):
                    xsq = xp.tile([128, 512], bf, tag="ln_xsq")
                    nc.vector.tensor_mul(xsq[:, :cn], src[:, kt, cs:cs + cn],
                                         src[:, kt, cs:cs + cn])
                    mm(ssq[:, 1, :cn], ones_c, xsq[:, :cn],
                       start=(kt == 0), stop=(kt == NKT - 1))
                nc.vector.tensor_scalar_mul(mean_f[:, cs:cs + cn],
                                            ssq[:, 0, :cn], 1.0 / E)
                m2c = tmp.tile([1, 512], f32, tag="ln_m2c")
                nc.vector.tensor_mul(m2c[:, :cn], mean_f[:, cs:cs + cn],
                                     mean_f[:, cs:cs + cn])
                nc.vector.scalar_tensor_tensor(
                    var_f[:, cs:cs + cn], ssq[:, 1, :cn], 1.0 / E,
                    m2c[:, :cn], op0=OP.mult, op1=OP.subtract)
            nc.scalar.activation(std_f, var_f, AF.Sqrt, bias=eps_t)
            nc.vector.reciprocal_approx_fast(rstd_f, std_f)
            nc.vector.tensor_copy(m_bf, mean_f)
            nc.vector.tensor_copy(r_bf, rstd_f)
            nc.gpsimd.partition_broadcast(r_bc, r_bf)
            st["m_bf"] = m_bf
            st["r_bc"] = r_bc
            st["std_f"] = std_f
            if want_cols:
                rcol = rp.tile([128, NTT], f32, tag="rcol")
                nc.vector.memset(rcol, 0.0)
                for it, (ts, tn) in enumerate(TOKTILES):
                    tp = pp.tile([128, 1], f32, tag="ln_tp")
                    nc.tensor.transpose(tp[:tn, :], rstd_f[:, ts:ts + tn],
                                        ident[:1, :1])
                    nc.vector.tensor_copy(rcol[:tn, it:it + 1], tp[:tn, :])
                st["rcol"] = rcol
            return st

        def dbg(name, tile):
            if DEBUG:
                nc.sync.dma_start(io[name], tile)

        # ========== phase 1: LN1-folded qkv / gates / lora / v / attention ======
        with tc.tile_pool(name="lnt1", bufs=1) as lnt1, \
             tc.tile_pool(name="lnx1", bufs=2) as lnx1, \
             tc.tile_pool(name="ppLN", bufs=2, space="PSUM") as ppLN:
            st1 = ln_stats(x_sb, rp1, lnt1, lnx1, ppLN, want_cols=True,
                           keep_std=cfg["has_bh"])
            dbg("dbg_mbf", st1["m_bf"])
            dbg("dbg_rbc", st1["r_bc"])
            dbg("dbg_rcol", st1["rcol"])

        # v/qk/proj weight DMAs early (wv reuses lnt1's space, freed quickly)
        wvctx = contextlib.ExitStack()
        wvp = wvctx.enter_context(tc.tile_pool(name="wvp", bufs=1))
        wv_sb = wvp.tile([128, NKT, E], bf)
        nc.sync.dma_start(wv_sb, io["wv"].rearrange("(k p) m -> p k m", p=128))
        wqkr = io["wqk"].rearrange("(k p) m -> p k m", p=128)
        for mlo, mhi in [(0, 512), (1024, 1536), (512, 1024), (1536, 2048)]:
            nc.sync.dma_start(wqk_sb[:, :, mlo:mhi], wqkr[:, :, mlo:mhi])
        nc.sync.dma_start(wp_sb, io["wp"].rearrange("(k p) m -> p k m", p=128))

        # gates + lora h (combined atz chain: rows 0-63 = A, 64-67 = gate)
        with tc.tile_pool(name="gt1", bufs=1) as gt1, \
             tc.tile_pool(name="ppG", bufs=2, space="PSUM") as ppG:
            for (cs, cn) in CHUNKS:
                hz_ps = ppG.tile([68, 512], f32, tag="hz")
                for kt in range(NKT - 1):
                    mm(hz_ps[:, :cn], atz_sb[:, kt, :],
                       x_sb[:, kt, cs:cs + cn], start=(kt == 0), stop=False)
                mm(hz_ps[:, :cn], negrs_sb, st1["m_bf"][:, cs:cs + cn],
                   start=False, stop=False)
                mm(hz_ps[:, :cn], atz_sb[:, NKT - 1, :],
                   x_sb[:, NKT - 1, cs:cs + cn], start=False, stop=True)
                # gate softmax from rows 64:68 (needs r scaling)
                zt = gt1.tile([4, 512], f32, tag="g_zt")
                nc.vector.tensor_mul(zt[:, :cn], hz_ps[64:68, :cn],
                                     st1["r_bc"][0:4, cs:cs + cn])
                if cfg["has_bgl"]:
                    nc.vector.tensor_scalar_add(zt[:, :cn], zt[:, :cn],
                                                bgl_sb)
                ez = gt1.tile([4, 512], bf, tag="g_ez")
                nc.scalar.activation(ez[:, :cn], zt[:, :cn], AF.Exp)
                den = ppG.tile([1, 512], f32, tag="g_den")
                mm(den[:, :cn], ones_c[0:4, :], ez[:, :cn],
                   start=True, stop=True)
                gdr = gt1.tile([1, 2, 512], f32, tag="g_dr")
                nc.vector.tensor_copy(gdr[:, 0, :cn], den[:, :cn])
                rden = gdr[:, 1, :]
                nc.vector.reciprocal_approx_fast(rden[:, :cn], gdr[:, 0, :cn])
                rdb = gt1.tile([4, 512], f32, tag="g_rdb")
                nc.gpsimd.partition_broadcast(rdb[:, :cn], rden[:, :cn])
                g_lora = gt1.tile([4, 512], bf, tag="g_lora")
                nc.vector.tensor_mul(g_lora[:, :cn], ez[:, :cn], rdb[:, :cn])
                if DEBUG:
                    nc.sync.dma_start(io["dbg_g"][:, cs:cs + cn], g_lora[:, :cn])
                # h_u (un-r-scaled lora down) -> gh_u = g * h_u
                h_sb = gt1.tile([64, 512], f32, tag="h_sb")
                nc.vector.tensor_copy(h_sb[:, :cn], hz_ps[0:64, :cn])
                if cfg["has_bh"]:
                    # h_true = r*h_u + bh  =>  h_u' = h_u + bh*std
                    sbc = gt1.tile([64, 512], f32, tag="sbc")
                    nc.gpsimd.partition_broadcast(sbc[:, :cn],
                                                  st1["std_f"][:, cs:cs + cn])
                    nc.vector.scalar_tensor_tensor(
                        sbc[:, :cn], sbc[:, :cn], bh_sb, h_sb[:, :cn],
                        op0=OP.mult, op1=OP.add)
                    h_sb = sbc
                ge_ps = ppG.tile([64, 512], f32, tag="ge")
                mm(ge_ps[:, :cn], elora_sb, g_lora[:, :cn],
                   start=True, stop=True)
                nc.vector.tensor_mul(gh_lora[:, cs:cs + cn], h_sb[:, :cn],
                                     ge_ps[:, :cn])

        dbg("dbg_gh", gh_lora)
        # ---------------- v (token-major, ones-interleaved) ----------------
        with tc.tile_pool(name="pp_v", bufs=2, space="PSUM") as pp_v:
            for it, (ts, tn) in enumerate(TOKTILES):
                for oc in (0, 512):
                    ps = pp_v.tile([128, 512], f32, tag="v_ps")
                    for kt in range(NKT - 1):
                        mm(ps[:tn], x_sb[:, kt, ts:ts + tn],
                           wv_sb[:, kt, oc:oc + 512], start=(kt == 0),
                           stop=False)
                    mm(ps[:tn], st1["m_bf"][:, ts:ts + tn],
                       negcsv_sb[:, oc:oc + 512], start=False, stop=False)
                    mm(ps[:tn], gh_lora[:, ts:ts + tn],
                       bqkvT_sb[:, 2048 + oc:2048 + oc + 512],
                       start=False, stop=False)
                    mm(ps[:tn], x_sb[:, NKT - 1, ts:ts + tn],
                       wv_sb[:, NKT - 1, oc:oc + 512], start=False, stop=True)
                    dst = v_sb[:tn, it, :].rearrange("p (h c) -> p h c", c=65)[
                        :, oc // 64:oc // 64 + 8, 0:64]
                    src = ps[:tn, :].rearrange("p (h c) -> p h c", c=64)
                    nc.vector.tensor_scalar_mul(
                        dst, src, st1["rcol"][:tn, it:it + 1])
                    if cfg["has_bv"]:
                        bvv = bv_bc[:tn, oc:oc + 512].rearrange(
                            "p (h c) -> p h c", c=64)
                        nc.vector.tensor_add(dst, dst, bvv)
        dbg("dbg_v", v_sb[:, 0, :])
        wvctx.close()

        # -------- interleaved qk Mtiles + attention head pairs --------
        with tc.tile_pool(name="pe_", bufs=6) as pe_, \
             tc.tile_pool(name="psm", bufs=2) as psm, \
             tc.tile_pool(name="psr", bufs=1) as psr, \
             tc.tile_pool(name="pp_qk", bufs=2, space="PSUM") as pp_qk, \
             tc.tile_pool(name="pp_s", bufs=2, space="PSUM") as pp_s, \
             tc.tile_pool(name="pp_cx", bufs=2, space="PSUM") as pp_cx:

            def qk_mt(mt):
                for (cs, cn) in CHUNKS:
                    ps = pp_qk.tile([128, 512], f32, tag="qk_ps")
                    for kt in range(NKT - 1):
                        mm(ps[:, :cn], wqk_sb[:, kt, mt * 128:(mt + 1) * 128],
                           x_sb[:, kt, cs:cs + cn], start=(kt == 0),
                           stop=False)
                    mm(ps[:, :cn], negcsqk_sb[:, mt * 128:(mt + 1) * 128],
                       st1["m_bf"][:, cs:cs + cn], start=False, stop=False)
                    mm(ps[:, :cn], bqkvT_sb[:, mt * 128:(mt + 1) * 128],
                       gh_lora[:, cs:cs + cn], start=False, stop=False)
                    mm(ps[:, :cn], wqk_sb[:, NKT - 1, mt * 128:(mt + 1) * 128],
                       x_sb[:, NKT - 1, cs:cs + cn], start=False, stop=True)
                    nc.vector.tensor_mul(qk_sb[:, mt, cs:cs + cn], ps[:, :cn],
                                         st1["r_bc"][:, cs:cs + cn])
                    if cfg["has_bqk"]:
                        nc.vector.tensor_scalar_add(
                            qk_sb[:, mt, cs:cs + cn], qk_sb[:, mt, cs:cs + cn],
                            bqk_sb[:, mt:mt + 1])

            def attn_unit_A(j, b, qs, qn):
                h0, h1 = 2 * j, 2 * j + 1
                mtq, mtk = j, 8 + j
                g0 = b * N + qs
                e_tiles = []
                for kt in range(NQT):
                    ks = b * N + kt * 128
                    ksz = min(128, N - kt * 128)
                    s = pp_s.tile([128, 2, 512], f32, tag="s")
                    mm(s[:ksz, 0, :qn], qk_sb[0:64, mtk, ks:ks + ksz],
                       qk_sb[0:64, mtq, g0:g0 + qn], start=True, stop=True)
                    mm(s[:ksz, 1, :qn], qk_sb[64:128, mtk, ks:ks + ksz],
                       qk_sb[64:128, mtq, g0:g0 + qn], start=True, stop=True)
                    e = pe_.tile([128, 2, 512], bf, tag="e")
                    if qn == 512:
                        nc.scalar.activation(e[:ksz], s[:ksz],
                                             AF.Exp, scale=HD ** -0.5)
                    else:
                        nc.scalar.activation(e[:ksz, 0, :qn], s[:ksz, 0, :qn],
                                             AF.Exp, scale=HD ** -0.5)
                        nc.scalar.activation(e[:ksz, 1, :qn], s[:ksz, 1, :qn],
                                             AF.Exp, scale=HD ** -0.5)
                    e_tiles.append((e, ksz))
                cx0 = pp_cx.tile([65, 512], f32, tag="cx")
                for kt, (e, ksz) in enumerate(e_tiles):
                    mm(cx0[:, :qn],
                       v_sb[:ksz, b * NQT + kt, h0 * 65:h0 * 65 + 65],
                       e[:ksz, 0, :qn], start=(kt == 0), stop=(kt == NQT - 1))
                r01 = psm.tile([1, 2, 512], f32, tag="r01")
                dcp = psr.tile([1, 2, 512], f32, tag="dcp")
                nc.vector.tensor_copy(dcp[:, 0, :qn], cx0[64:65, :qn])
                nc.vector.reciprocal_approx_fast(r01[:, 0, :qn],
                                                 dcp[:, 0, :qn])
                cx1 = pp_cx.tile([65, 512], f32, tag="cx")
                for kt, (e, ksz) in enumerate(e_tiles):
                    mm(cx1[:, :qn],
                       v_sb[:ksz, b * NQT + kt, h1 * 65:h1 * 65 + 65],
                       e[:ksz, 1, :qn], start=(kt == 0), stop=(kt == NQT - 1))
                nc.vector.tensor_copy(dcp[:, 1, :qn], cx1[64:65, :qn])
                nc.vector.reciprocal_approx_fast(r01[:, 1, :qn],
                                                 dcp[:, 1, :qn])
                return (j, g0, qn, cx0, cx1, r01)

            def attn_unit_B(st):
                j, g0, qn, cx0, cx1, r01 = st
                Rs = psr.tile([64, 2, 512], f32, tag="Rs")
                nc.gpsimd.partition_broadcast(Rs[:, 0, :qn], r01[:, 0, :qn])
                nc.gpsimd.partition_broadcast(Rs[:, 1, :qn], r01[:, 1, :qn])
                nc.vector.tensor_mul(ctx_sb[0:64, j, g0:g0 + qn],
                                     cx0[0:64, :qn], Rs[:, 0, :qn])
                nc.vector.tensor_mul(ctx_sb[64:128, j, g0:g0 + qn],
                                     cx1[0:64, :qn], Rs[:, 1, :qn])

            pending = [None]

            def attn_pair(j):
                # big q-chunks first so unit_B latency hides under scores
                for (qs, qn) in QCHUNKS:
                    for b in range(BLOC):
                        st = attn_unit_A(j, b, qs, qn)
                        if pending[0] is not None:
                            attn_unit_B(pending[0])
                        pending[0] = st

            for j in range(H // 2):
                qk_mt(j)
                if DEBUG:
                    nc.sync.dma_start(io["dbg_qkm"][:, j, :], qk_sb[:, j, :])
                if j == 0:
                    dbg("dbg_qk", qk_sb[:, 0, :])
                qk_mt(8 + j)
                if DEBUG:
                    nc.sync.dma_start(io["dbg_qkm"][:, 8 + j, :],
                                      qk_sb[:, 8 + j, :])
                attn_pair(j)
            attn_unit_B(pending[0])
            dbg("dbg_ctx", ctx_sb)
            dbg("dbg_qkf", qk_sb)

        wqkctx.close()
        p1ctx.close()

        # ========= phase 2: proj + residual, chunk-pipelined LN2 stats =========
        with tc.tile_pool(name="lnt2", bufs=1) as lnt2, \
             tc.tile_pool(name="lnx2", bufs=2) as lnx2, \
             tc.tile_pool(name="pp_pr", bufs=3, space="PSUM") as pp_pr, \
             tc.tile_pool(name="ppLN2", bufs=2, space="PSUM") as ppLN2:
            rows2 = lnt2.tile([1, 3, T], f32, tag="ln2_trows")
            mean2 = rows2[:, 0, :]
            var2 = rows2[:, 1, :]
            std2 = rows2[:, 2, :]
            rstd2 = var2                            # recip overwrites var
            r2_bc = lnt2.tile([128, T], bf, tag="r2_bc")
            mr2_bc = lnt2.tile([128, T], bf, tag="mr2_bc")
            brow2 = lnt2.tile([1, 2, T], bf, tag="ln2_brows")
            r2_bf = brow2[:, 0, :]
            mr2_bf = brow2[:, 1, :]
            def proj_chunk(ci):
                cs, cn = CHUNKS[ci]
                for mt in range(NKT):
                    ps = pp_pr.tile([128, 512], f32, tag="pr_ps")
                    for kt in range(NKT):
                        mm(ps[:, :cn], wp_sb[:, kt, mt * 128:(mt + 1) * 128],
                           ctx_sb[:, kt, cs:cs + cn],
                           start=(kt == 0), stop=(kt == NKT - 1))
                    nc.vector.scalar_tensor_tensor(
                        x_sb[:, mt, cs:cs + cn], ps[:, :cn],
                        bp_sb[:, mt:mt + 1], x_sb[:, mt, cs:cs + cn],
                        op0=OP.add, op1=OP.add)

            def ln2_chunk(ci):
                cs, cn = CHUNKS[ci]
                ssq = ppLN2.tile([1, 2, 512], f32, tag="ln2_rows")
                for kt in range(NKT):
                    mm(ssq[:, 0, :cn], ones_c, x_sb[:, kt, cs:cs + cn],
                       start=(kt == 0), stop=(kt == NKT - 1))
                for kt in range(NKT):
                    xsq = lnx2.tile([128, 512], bf, tag="ln2_xsq")
                    nc.vector.tensor_mul(xsq[:, :cn], x_sb[:, kt, cs:cs + cn],
                                         x_sb[:, kt, cs:cs + cn])
                    mm(ssq[:, 1, :cn], ones_c, xsq[:, :cn],
                       start=(kt == 0), stop=(kt == NKT - 1))
                nc.vector.tensor_scalar_mul(mean2[:, cs:cs + cn], ssq[:, 0, :cn],
                                            1.0 / E)
                m2c = lnt2.tile([1, 512], f32, tag="ln2_m2c")
                nc.vector.tensor_mul(m2c[:, :cn], mean2[:, cs:cs + cn],
                                     mean2[:, cs:cs + cn])
                nc.vector.scalar_tensor_tensor(
                    var2[:, cs:cs + cn], ssq[:, 1, :cn], 1.0 / E,
                    m2c[:, :cn], op0=OP.mult, op1=OP.subtract)
                nc.scalar.activation(std2[:, cs:cs + cn], var2[:, cs:cs + cn],
                                     AF.Sqrt, bias=eps_t)
                nc.vector.reciprocal_approx_fast(rstd2[:, cs:cs + cn],
                                                 std2[:, cs:cs + cn])
                nc.vector.tensor_copy(r2_bf[:, cs:cs + cn],
                                      rstd2[:, cs:cs + cn])
                nc.vector.tensor_mul(mr2_bf[:, cs:cs + cn],
                                     mean2[:, cs:cs + cn],
                                     rstd2[:, cs:cs + cn])
                nc.gpsimd.partition_broadcast(r2_bc[:, cs:cs + cn],
                                              r2_bf[:, cs:cs + cn])
                nc.gpsimd.partition_broadcast(mr2_bc[:, cs:cs + cn],
                                              mr2_bf[:, cs:cs + cn])

            def norm_chunk(ci):
                cs, cn = CHUNKS[ci]
                for kt in range(NKT):
                    tmp = lnx2.tile([128, 512], bf, tag="ln2_tmp")
                    nc.vector.tensor_mul(tmp[:, :cn], x_sb[:, kt, cs:cs + cn],
                                         r2_bc[:, cs:cs + cn])
                    nc.vector.tensor_sub(n2[:, kt, cs:cs + cn], tmp[:, :cn],
                                         mr2_bc[:, cs:cs + cn])

            proj_chunk(0)
            proj_chunk(1)
            ln2_chunk(0)
            proj_chunk(2)
            ln2_chunk(1)
            ln2_chunk(2)
            norm_chunk(0)
            norm_chunk(1)
            norm_chunk(2)
        wpctx.close()
        actx.close()

        # ================= phase 3: MLP + adapter (fc quarters) =================
        sp3 = ctx.enter_context(tc.tile_pool(name="sp3", bufs=1))
        ead_sb = sp3.tile([4, 256], bf)
        nc.sync.dma_start(ead_sb, io["ead"])
        bfc2_sb = sp3.tile([128, 8], f32)
        nc.sync.dma_start(bfc2_sb, io["bfc2"].rearrange("(m p) -> p m", p=128))
        wad_sb = sp3.tile([128, NKT, 256], bf)
        nc.sync.dma_start(wad_sb, io["wad"].rearrange("(k p) c -> p k c", p=128))
        up_sb = sp3.tile([128, 2, E], bf)
        nc.sync.dma_start(
            up_sb, io["upaug"][0:256, :].rearrange("(k p) e -> p k e", p=128))
        up_tail = sp3.tile([4, E], bf)
        nc.sync.dma_start(up_tail, io["upaug"][256:260, :])
        if cfg["has_bfc1"]:
            bfc1_sb = sp3.tile([128, 32], f32)
            nc.sync.dma_start(bfc1_sb, io["bfc1"].rearrange("(m p) -> p m", p=128))
        if cfg["has_bad"]:
            bad_sb = sp3.tile([128, 2], f32)
            nc.sync.dma_start(bad_sb, io["bad"].rearrange("(m p) -> p m", p=128))
        partial = sp3.tile([128, NKT, T], bf, tag="partial")
        g_ad = sp3.tile([4, T], bf, tag="g_ad")

        wfc1_all = io["wfc1"].rearrange("(k p) m -> p k m", p=128)
        wfc2_all = io["wfc2"].rearrange("(k p) m -> p k m", p=128)
        outr = io["out_fm"].rearrange("(k p) t -> p k t", p=128)

        with tc.tile_pool(name="p3s", bufs=2) as p3s, \
             tc.tile_pool(name="p3c", bufs=2) as p3c, \
             tc.tile_pool(name="p3t", bufs=2) as p3t, \
             tc.tile_pool(name="pp_f1", bufs=2, space="PSUM") as pp_f1, \
             tc.tile_pool(name="pp_f2", bufs=2, space="PSUM") as pp_f2, \
             tc.tile_pool(name="pp_ge", bufs=2, space="PSUM") as pp_ge:
            # adapter gates (explicit n2; fills the fc-weight DMA wait)
            for (cs, cn) in CHUNKS:
                z_ps = pp_f1.tile([4, 512], f32, tag="f1_ps")
                for kt in range(NKT):
                    mm(z_ps[:, :cn], wgad_sb[:, kt, :], n2[:, kt, cs:cs + cn],
                       start=(kt == 0), stop=(kt == NKT - 1))
                zt = p3t.tile([4, 512], f32, tag="g2_zt")
                if cfg["has_bgad"]:
                    nc.vector.tensor_scalar_add(zt[:, :cn], z_ps[:, :cn],
                                                bgad_sb)
                else:
                    nc.vector.tensor_copy(zt[:, :cn], z_ps[:, :cn])
                ez = p3t.tile([4, 512], bf, tag="g2_ez")
                nc.scalar.activation(ez[:, :cn], zt[:, :cn], AF.Exp)
                den = pp_f2.tile([1, 512], f32, tag="f2_ps")
                mm(den[:, :cn], ones_c[0:4, :], ez[:, :cn],
                   start=True, stop=True)
                dcp2 = p3t.tile([1, 512], f32, tag="g2_dcp")
                nc.vector.tensor_copy(dcp2[:, :cn], den[:, :cn])
                rden = p3t.tile([1, 512], f32, tag="g2_rden")
                nc.vector.reciprocal_approx_fast(rden[:, :cn], dcp2[:, :cn])
                rdb = p3t.tile([4, 512], f32, tag="g2_rdb")
                nc.gpsimd.partition_broadcast(rdb[:, :cn], rden[:, :cn])
                nc.vector.tensor_mul(g_ad[:, cs:cs + cn], ez[:, :cn],
                                     rdb[:, :cn])
            for q in range(NFQ):
                wfc1_q = p3s.tile([128, NKT, FH * 128], bf, tag="wfc1q")
                for half in range(2):
                    h0, h1 = half * FH * 64, (half + 1) * FH * 64
                    nc.sync.dma_start(
                        wfc1_q[:, :, h0:h1],
                        wfc1_all[:, :, q * FH * 128 + h0:q * FH * 128 + h1])
                wfc2_q = p3s.tile([128, FH, E], bf, tag="wfc2q")
                for half in range(2):
                    nc.sync.dma_start(
                        wfc2_q[:, half * FH // 2:(half + 1) * FH // 2, :],
                        wfc2_all[:, q * FH + half * FH // 2:
                                 q * FH + (half + 1) * FH // 2, :])
                last = (q == NFQ - 1)
                chunk_order = [0, 1, 2]
                for ci in chunk_order:
                    cs, cn = CHUNKS[ci]
                    if last:
                        # adapter: gated gelu bottleneck
                        gah = p3t.tile([128, 2, 512], bf, tag="gah")
                        ad_ps = pp_f1.tile([128, 2, 512], f32, tag="f1_ps")
                        for amt in range(2):
                            for kt in range(NKT):
                                mm(ad_ps[:, amt, :cn],
                                   wad_sb[:, kt, amt * 128:(amt + 1) * 128],
                                   n2[:, kt, cs:cs + cn],
                                   start=(kt == 0), stop=(kt == NKT - 1))
                        ah = p3t.tile([128, 2, 512], bf, tag="ah")
                        for amt in range(2):
                            if cfg["has_bad"]:
                                nc.scalar.activation(
                                    ah[:, amt, :cn], ad_ps[:, amt, :cn], AF.Gelu,
                                    bias=bad_sb[:, amt:amt + 1])
                            else:
                                nc.scalar.activation(ah[:, amt, :cn],
                                                     ad_ps[:, amt, :cn], AF.Gelu)
                        for amt in range(2):
                            ge = pp_ge.tile([128, 512], f32, tag="ge2")
                            mm(ge[:, :cn], ead_sb[:, amt * 128:(amt + 1) * 128],
                               g_ad[:, cs:cs + cn], start=True, stop=True)
                            nc.vector.tensor_mul(gah[:, amt, :cn],
                                                 ah[:, amt, :cn], ge[:, :cn])
                    # fc1 -> gelu -> h1 (this quarter), Mtiles in pairs
                    h1 = p3c.tile([128, FH, 512], bf, tag="h1")
                    for mtp in range(FH // 2):
                        ps = pp_f1.tile([128, 2, 512], f32, tag="f1_ps")
                        for sub in range(2):
                            mt = mtp * 2 + sub
                            for kt in range(NKT):
                                mm(ps[:, sub, :cn],
                                   wfc1_q[:, kt, mt * 128:(mt + 1) * 128],
                                   n2[:, kt, cs:cs + cn],
                                   start=(kt == 0), stop=(kt == NKT - 1))
                        for sub in range(2):
                            if cfg["has_bfc1"]:
                                gmt = q * FH + mtp * 2 + sub
                                nc.scalar.activation(
                                    h1[:, mtp * 2 + sub, :cn], ps[:, sub, :cn],
                                    AF.Gelu, bias=bfc1_sb[:, gmt:gmt + 1])
                            else:
                                nc.scalar.activation(
                                    h1[:, mtp * 2 + sub, :cn], ps[:, sub, :cn],
                                    AF.Gelu)
                    # fc2 quarter (+ adapter-up merged into last quarter)
                    for mt in range(NKT):
                        ps2 = pp_f2.tile([128, 512], f32, tag="f2_ps")
                        for kt in range(FH):
                            mm(ps2[:, :cn],
                               wfc2_q[:, kt, mt * 128:(mt + 1) * 128],
                               h1[:, kt, :cn], start=(kt == 0),
                               stop=(kt == FH - 1 and not last))
                        if q == 0:
                            nc.vector.tensor_copy(partial[:, mt, cs:cs + cn],
                                                  ps2[:, :cn])
                        elif not last:
                            nc.vector.tensor_add(partial[:, mt, cs:cs + cn],
                                                 ps2[:, :cn],
                                                 partial[:, mt, cs:cs + cn])
                        else:
                            for akt in range(2):
                                mm(ps2[:, :cn],
                                   up_sb[:, akt, mt * 128:(mt + 1) * 128],
                                   gah[:, akt, :cn], start=False, stop=False)
                            mm(ps2[:, :cn], up_tail[:, mt * 128:(mt + 1) * 128],
                               g_ad[:, cs:cs + cn], start=False, stop=True)
                            ot = p3t.tile([128, 512], f32, tag="ot")
                            nc.vector.scalar_tensor_tensor(
                                ot[:, :cn], ps2[:, :cn], bfc2_sb[:, mt:mt + 1],
                                partial[:, mt, cs:cs + cn],
                                op0=OP.add, op1=OP.add)
                            ob = p3t.tile([128, 512], bf, tag="ob")
                            nc.vector.tensor_add(ob[:, :cn], ot[:, :cn],
                                                 x_sb[:, mt, cs:cs + cn])
                            nc.sync.dma_start(outr[:, mt, cs:cs + cn],
                                              ob[:, :cn])


def _prep_weights(inputs):
    """Host-side weight preparation (LN folding, transposes, bf16 casts)."""
    f = np.float32
    g1 = np.asarray(inputs["ln1_g"], f)
    b1 = np.asarray(inputs["ln1_b"], f)
    g2 = np.asarray(inputs["ln2_g"], f)
    b2 = np.asarray(inputs["ln2_b"], f)
    qkv_w = np.asarray(inputs["qkv_w"], f)
    Wq = qkv_w * g1[None, :]
    bqkv = np.asarray(inputs["qkv_b"], f) + qkv_w @ b1
    A = np.asarray(inputs["lora_A"], f)
    Afold = (A * g1[None, None, :]).reshape(LORA_E * LORA_R, E)
    Bm = np.asarray(inputs["lora_B"], f)
    lgw = np.asarray(inputs["lora_gate_w"], f)
    lgw_fold = lgw * g1[None, :]
    fc1_w = np.asarray(inputs["fc1_w"], f)
    fc2_w = np.asarray(inputs["fc2_w"], f)
    adg = np.asarray(inputs["ad_gate_w"], f)
    add_w = np.asarray(inputs["ad_down_w"], f).reshape(AD_E * AD_D, E)
    adu_w = np.asarray(inputs["ad_up_w"], f)

    elora = np.zeros((LORA_E, LORA_E * LORA_R), f)
    for x in range(LORA_E):
        elora[x, x * LORA_R:(x + 1) * LORA_R] = 1.0
    ead = np.zeros((AD_E, AD_E * AD_D), f)
    for x in range(AD_E):
        ead[x, x * AD_D:(x + 1) * AD_D] = 1.0

    bqk = bqkv[:2 * E]
    bv = bqkv[2 * E:]
    bh = (A.reshape(64, E) @ b1).astype(f)
    bgl = lgw @ b1
    bgad = adg @ b2
    bfc1 = (np.asarray(inputs["fc1_b"], f) + fc1_w @ b2).astype(f)
    bad = (np.asarray(inputs["ad_down_b"], f).reshape(AD_E * AD_D)
           + add_w @ b2).astype(f)
    atz = np.concatenate([Afold, lgw_fold], axis=0)   # [68, E]
    w = {
        "wqk": np.ascontiguousarray(Wq[:2 * E].T).astype(BF16),
        "wv": np.ascontiguousarray(Wq[2 * E:].T).astype(BF16),
        "negcsqk": np.ascontiguousarray(-Wq[:2 * E].sum(1)[None, :]).astype(BF16),
        "negcsv": np.ascontiguousarray(-Wq[2 * E:].sum(1)[None, :]).astype(BF16),
        "atz": np.ascontiguousarray(atz.T).astype(BF16),
        "negrs": np.ascontiguousarray(-atz.sum(1)[None, :]).astype(BF16),
        "bqk": np.ascontiguousarray(bqk),
        "bv": bv.astype(f),
        "bh": bh,
        "bgl": bgl.astype(f),
        "bqkvT": np.ascontiguousarray(
            np.transpose(Bm, (0, 2, 1)).reshape(64, 3 * E)).astype(BF16),
        "elora": elora.astype(BF16),
        "ead": ead.astype(BF16),
        "wp": np.ascontiguousarray(np.asarray(inputs["proj_w"], f).T).astype(BF16),
        "bp": np.asarray(inputs["proj_b"], f),
        "wfc1": np.ascontiguousarray((fc1_w * g2[None, :]).T).astype(BF16),
        "bfc1": bfc1,
        "wfc2": np.ascontiguousarray(fc2_w.T).astype(BF16),
        "bfc2": np.asarray(inputs["fc2_b"], f),
        "wgad": np.ascontiguousarray((adg * g2[None, :]).T).astype(BF16),
        "bgad": bgad.astype(f),
        "wad": np.ascontiguousarray((add_w * g2[None, :]).T).astype(BF16),
        "bad": bad,
        "upaug": np.concatenate(
            [np.transpose(adu_w, (0, 2, 1)).reshape(AD_E * AD_D, E),
             np.asarray(inputs["ad_up_b"], f)], axis=0).astype(BF16),
    }
    cfg = {
        "has_bqk": bool(np.abs(bqk).max() > 0),
        "has_bv": bool(np.abs(bv).max() > 0),
        "has_bh": bool(np.abs(bh).max() > 0),
        "has_bgl": bool(np.abs(bgl).max() > 0),
        "has_bgad": bool(np.abs(bgad).max() > 0),
        "has_bfc1": bool(np.abs(bfc1).max() > 0),
        "has_bad": bool(np.abs(bad).max() > 0),
    }
    return w, cfg


DEBUG = False

_CACHE = {}


def _get_program(cfg):
    key = tuple(sorted(cfg.items()))
    if key in _CACHE:
        return _CACHE[key]
    from concourse import bacc
    import concourse.tile as tile
    import concourse.mybir as mybir

    nc = bacc.Bacc("TRN2", target_bir_lowering=False, debug=False,
                   enable_asserts=False, num_devices=NCORES)
    f32 = mybir.dt.float32
    bf = mybir.dt.bfloat16
    shapes = {
        "x_fm": ([E, T], bf),
        "wqk": ([E, 2 * E], bf), "wv": ([E, E], bf),
        "negcsqk": ([1, 2 * E], bf), "negcsv": ([1, E], bf),
        "atz": ([E, 68], bf), "negrs": ([1, 68], bf),
        "bqk": ([2 * E], f32), "bv": ([E], f32), "bh": ([64], f32),
        "bgl": ([4], f32),
        "bqkvT": ([64, 3 * E], bf),
        "elora": ([4, 64], bf), "ead": ([4, 256], bf),
        "wp": ([E, E], bf), "bp": ([E], f32),
        "wfc1": ([E, FF], bf), "bfc1": ([FF], f32),
        "wfc2": ([FF, E], bf), "bfc2": ([E], f32),
        "wgad": ([E, 4], bf), "bgad": ([4], f32),
        "wad": ([E, 256], bf), "bad": ([256], f32),
        "upaug": ([260, E], bf),
    }
    skip = set()
    for flag, name in [("has_bqk", "bqk"), ("has_bv", "bv"), ("has_bh", "bh"),
                       ("has_bgl", "bgl"), ("has_bgad", "bgad"),
                       ("has_bfc1", "bfc1"), ("has_bad", "bad")]:
        if not cfg[flag]:
            skip.add(name)
    io = {}
    for name, (shape, dt) in shapes.items():
        if name in skip:
            continue
        io[name] = nc.dram_tensor(name, shape, dt, kind="ExternalInput").ap()
    io["out_fm"] = nc.dram_tensor("out_fm", [E, T], bf,
                                  kind="ExternalOutput").ap()
    if DEBUG:
        for nm, shape, dt in [("dbg_mbf", [1, T], bf), ("dbg_rbc", [128, T], bf),
                              ("dbg_rcol", [128, NTT], f32),
                              ("dbg_gh", [64, T], bf), ("dbg_qk", [128, T], bf),
                              ("dbg_v", [128, H * 65], bf),
                              ("dbg_g", [4, T], bf),
                              ("dbg_ctx", [128, NKT, T], bf),
                              ("dbg_qkf", [128, 16, T], bf),
                              ("dbg_qkm", [128, 16, T], bf),
                              ("dbg_n2", [128, T], bf),
                              ("dbg_t1", [128, T], bf),
                              ("dbg_r2rows", [1, 3, T], f32),
                              ("dbg_r2bc", [128, T], bf),
                              ("dbg_t0", [128, T], bf),
                              ("dbg_wp", [128, E], bf)]:
            io[nm] = nc.dram_tensor(nm, shape, dt, kind="ExternalOutput").ap()
    with tile.TileContext(nc) as tc:
        _build(tc, io, cfg)
    nc.compile()
    _CACHE[key] = (nc, {n for n in io if not n.startswith("dbg_") and n != "out_fm"})
    return _CACHE[key]


def kernel(**inputs):
    from concourse import bass_utils

    w, cfg = _prep_weights(inputs)
    nc, in_names = _get_program(cfg)

    tokens = np.asarray(inputs["tokens"], np.float32)
    in_maps = []
    for c in range(NCORES):
        m = {k: v for k, v in w.items() if k in in_names}
        x = tokens[c * BLOC:(c + 1) * BLOC].reshape(T, E).T
        m["x_fm"] = np.ascontiguousarray(x).astype(BF16)
        in_maps.append(m)

    res = bass_utils.run_bass_kernel_spmd(nc, in_maps, core_ids=list(range(NCORES)))
    out = np.empty((B, N, E), np.float32)
    for c in range(NCORES):
        of = np.asarray(res.results[c]["out_fm"], np.float32)
        out[c * BLOC:(c + 1) * BLOC] = of.T.reshape(BLOC, N, E)
    return out
